# revision 8
# baseline (speedup 1.0000x reference)
"""DTranNER CRF loss kernel for Trainium2 (8 NeuronCores, data-parallel over batch).

Strategy
--------
Batch (B=256) sharded 8 ways (BC=32 sentences/core).  Partition layout
(i, b): i = k-block (3 blocks of 8 states), b = sentence -> 96 partitions.

Pairwise CRF log-partition (the dominant cost, 511 streamed [24,24]
matrices/sentence) runs as fwd + bwd chains that meet in the middle, in
factored linear space (exp pre-scaled by CP, scale logs accumulated every
R=32 steps).  Per chain-step:
  * one DVE bf16 multiply [96, (24 n, 8 k)] against the SBUF bf16 state
    slice (hits the 2x DVE mode),
  * 24 tiny accumulating PE matmuls (0/1 selector lhsT) that perform the
    k-contraction AND the cross-partition redistribution in one pass,
    writing the next sliced state [96, 8] into PSUM,
  * a deferred DVE copy (PSUM -> SBUF bf16) issued at the START of the
    next slot so the PE+PSUM-write latency hides under the other chains'
    DVE work.  Lazy renorm: the copy doubles as scalar_tensor_tensor with
    accum_out (z partials) and per-partition 1/z scale.
Unary CRF chain: [64, BC] state, constant exp(trans) block matrix on PE,
one DVE multiply per slot.  Gold-path scores: host-gathered operands
(layout-only), summed on device; all per-b tail results are assembled
partition-major via tiny PE transpose-matmuls (no DRAM bounces).

HBM stream (37.7 MB feats_pp per core) is DMA'd and exp'd (ACT, fp32 ->
bf16) in 4-step sub-chunks of 20-step tiles for fine-grained pipelining;
constant uploads are packed into single DMAs to keep the DGE issue queue
short at startup, and the renorm-log postprocessing overlaps the final
slots.
"""

import numpy as np
import ml_dtypes
from contextlib import ExitStack

import concourse.bass as bass
import concourse.bacc as bacc
import concourse.tile as tile
from concourse import mybir
from concourse.bass_utils import run_bass_kernel_spmd

FP = mybir.dt.float32
BF = mybir.dt.bfloat16

B, T, K = 256, 512, 24
START, STOP = 22, 23
NCORES = 8
NS, KB = 3, 8          # K = NS*KB k-block split
P = NS * 32            # 96 partitions (i-major: p = i*32 + b)
UROW = 64

AF = mybir.ActivationFunctionType
ALU = mybir.AluOpType
AX = mybir.AxisListType


def build_kernel(BC=32, TT=512, TC=20, R=32, SPLIT=False, COPY="dve", UCOPY=False, CHOP=5, FIRST=0, SBUFS=3, BIGB=2, EBIGB=3):
    assert BC == 32
    NF2 = K * KB       # 192
    H = TT // 2        # fwd pairwise steps (matrices t = 0..H-1)
    HB = TT - 1 - H    # bwd steps (matrices t = TT-2..H, transposed)
    SL = H             # unary slots
    CP = 3.8           # pairwise exp pre-scale
    CU = 3.8           # unary exp pre-scale
    NR = (H + R - 1) // R + 1   # renorm z slots per chain

    nc = bacc.Bacc("TRN2", target_bir_lowering=False)
    fppF = nc.dram_tensor("fppF", [NS, BC, H, NF2], FP, kind="ExternalInput")
    fppB = nc.dram_tensor("fppB", [NS, BC, HB, NF2], FP, kind="ExternalInput")
    winit = nc.dram_tensor("winit", [P, KB], FP, kind="ExternalInput")
    ftp2 = nc.dram_tensor("ftp2", [SL, UROW, BC], FP, kind="ExternalInput")
    eflast = nc.dram_tensor("eflast", [K, BC], FP, kind="ExternalInput")
    transPK = nc.dram_tensor("transPK", [UROW, K], FP, kind="ExternalInput")
    gvals = nc.dram_tensor("gvals", [BC, 3 * TT + 4], FP, kind="ExternalInput")
    selpack = nc.dram_tensor("selpack", [P, 32 + P + K], BF, kind="ExternalInput")
    nll = nc.dram_tensor("nll", [BC], FP, kind="ExternalOutput")
    scr = nc.dram_tensor("scratch", [4, BC], FP)

    with tile.TileContext(nc) as tc, ExitStack() as ctx:
        sb = ctx.enter_context(tc.tile_pool(name="sb", bufs=SBUFS))
        big = ctx.enter_context(tc.tile_pool(name="big", bufs=BIGB))
        ebig = ctx.enter_context(tc.tile_pool(name="ebig", bufs=EBIGB))
        per = ctx.enter_context(tc.tile_pool(name="per", bufs=1))
        psF = ctx.enter_context(tc.tile_pool(name="psF", bufs=2, space="PSUM"))
        psB = ctx.enter_context(tc.tile_pool(name="psB", bufs=2, space="PSUM"))
        psU = ctx.enter_context(tc.tile_pool(name="psU", bufs=2, space="PSUM"))
        psZ = ctx.enter_context(tc.tile_pool(name="psZ", bufs=1, space="PSUM"))
        ps1 = ctx.enter_context(tc.tile_pool(name="ps1", bufs=1, space="PSUM"))

        # ---------------- constants ----------------
        cpb = per.tile([128, 1], FP, tag="cpb", name="cpb")
        nc.vector.memset(cpb[:], -CP)

        cub = per.tile([128, 1], FP, tag="cub", name="cub")
        nc.vector.memset(cub[:], -CU)
        selpack_sb = per.tile([P, 32 + P + K], BF, tag="selpack", name="selpack_sb")
        nc.sync.dma_start(out=selpack_sb[:], in_=selpack[:])
        sel32_sb = selpack_sb[:, 0:32]
        sel3_sb = selpack_sb[:, 32 : 32 + P]
        selDN_sb = selpack_sb[0:UROW, 32 + P : 32 + P + K]

        # unary stationary weights (block matrix, as v1)
        uwst = per.tile([UROW, K], FP, tag="uwst", name="uwst")
        nc.sync.dma_start(out=uwst[:], in_=transPK[:])
        uw = per.tile([UROW, UROW], BF, tag="uw", name="uw")
        nc.vector.memset(uw[:], 0.0)
        nc.scalar.activation(out=uw[0:K, 0:K], in_=uwst[0:K, :], func=AF.Exp)
        nc.scalar.activation(
            out=uw[32 : 32 + K, 32 : 32 + K], in_=uwst[32 : 32 + K, :], func=AF.Exp
        )
        uones = per.tile([UROW, 2], BF, tag="uones", name="uones")
        nc.vector.memset(uones[:], 0.0)
        nc.vector.memset(uones[0:K, 0:1], 1.0)
        nc.vector.memset(uones[32 : 32 + K, 1:2], 1.0)
        usel = per.tile([2, UROW], BF, tag="usel", name="usel")
        nc.vector.memset(usel[:], 0.0)
        nc.vector.memset(usel[0:1, 0:K], 1.0)
        rowB = sb.tile([1, UROW], BF, tag="rowB", name="rowB")
        nc.vector.memset(rowB[:], 0.0)
        nc.vector.memset(rowB[0:1, 32 : 32 + K], 1.0)
        nc.sync.dma_start(out=usel[1:2, :], in_=rowB[:])
        ones2 = per.tile([2, 1], BF, tag="ones2", name="ones2")
        nc.vector.memset(ones2[:], 1.0)

        # ---------------- unary Ef table ----------------
        eft = per.tile([UROW, SL * BC], BF, tag="eft", name="eft")
        nchunk = 8
        cs2 = SL // nchunk
        cstep = cs2 * BC
        src = ftp2[:, :, :].rearrange("s r j -> r s j")

        def load_eft_chunk(c):
            ftile = big.tile([UROW, cstep], FP, tag="ftp_in", name="ftile_u")
            nc.sync.dma_start(
                out=ftile[:].rearrange("p (s j) -> p s j", j=BC),
                in_=src[:, c * cs2 : (c + 1) * cs2, :],
            )
            nc.scalar.activation(
                out=eft[:, c * cstep : (c + 1) * cstep], in_=ftile[:],
                func=AF.Exp, bias=cub[0:UROW, :],
            )

        load_eft_chunk(0)

        # ---------------- state init ----------------
        # fwd pairwise state: e_START sliced: START=22 -> i=2, k8=6
        uf0 = per.tile([P, KB], BF, tag="uf0", name="uf0")
        nc.vector.memset(uf0[:], 0.0)
        nc.vector.memset(uf0[64:96, 6:7], 1.0)

        # bwd pairwise init: exp(fpp[b, T-1, STOP, :] - CP), sliced (i b) k8
        wf = sb.tile([P, KB], FP, tag="wf", name="wf")
        nc.sync.dma_start(out=wf[:], in_=winit[:, :])
        ub0 = per.tile([P, KB], BF, tag="ub0", name="ub0")
        nc.scalar.activation(out=ub0[:], in_=wf[:], func=AF.Exp, bias=cpb[0:P, :])

        # pairwise renorm z-log buffer (PSUM, cols 0..NR-1 fwd, 40..40+NR bwd)
        zb = psZ.tile([P, 80], FP, tag="zb", name="zb")
        nc.vector.memset(zb[:], 1.0)

        # unary state [UROW, BC] + z buffer (as v1)
        us0 = per.tile([UROW, BC], BF, tag="us0", name="us0")
        nc.vector.memset(us0[:], 0.0)
        row1 = sb.tile([1, BC], BF, tag="row1", name="row1")
        nc.vector.memset(row1[:], 1.0)
        nc.sync.dma_start(out=us0[START : START + 1, :], in_=row1[:])
        tstop = sb.tile([UROW, 1], FP, tag="tstop", name="tstop")
        nc.sync.dma_start(
            out=tstop[32 : 32 + K, :],
            in_=transPK[32 + STOP : 32 + STOP + 1, :].rearrange("o k -> k o"),
        )
        tstop_e = sb.tile([UROW, 1], BF, tag="tstop_e", name="tstop_e")
        nc.scalar.activation(out=tstop_e[32 : 32 + K, :], in_=tstop[32 : 32 + K, :], func=AF.Exp)
        nc.vector.tensor_copy(
            out=us0[32 : 32 + K, :], in_=tstop_e[32 : 32 + K, :].broadcast_to([K, BC])
        )
        zbufU = per.tile([2, NR * BC], FP, tag="zbufU", name="zbufU")
        nc.vector.memset(zbufU[:], 1.0)



        # ---------------- helpers ----------------
        ones_pk = per.tile([P, KB], BF, tag="ones_pk", name="ones_pk")
        nc.vector.memset(ones_pk[:], 1.0)
        gv = per.tile([BC, 3 * TT + 4], FP, tag="gv", name="gv")
        sc_early = per.tile([BC, 1], FP, tag="sc_early", name="sc_early")
        lzb = per.tile([P, 80], FP, tag="lzb", name="lzb")
        sFe = per.tile([P, 1], FP, tag="sFe", name="sFe")
        sBe = per.tile([P, 1], FP, tag="sBe", name="sBe")

        def pair_mm(eX, m, st, ups, c0, tag):
            """Pairwise chain step compute: DVE mult + 24 accumulating PE matmuls
            into ups[:, c0:c0+KB]."""
            e3 = eX[:, m * NF2 : (m + 1) * NF2].rearrange("q (a b) -> q a b", a=K)
            prod = sb.tile([P, NF2], BF, tag=f"prod{tag}", name=f"prod{tag}")
            p3 = prod[:].rearrange("q (a b) -> q a b", a=K)
            nc.vector.tensor_tensor(out=p3, in0=e3, in1=st, op=ALU.mult)
            for ip in range(NS):
                for kk in range(KB):
                    rhs = p3[:, ip * KB : (ip + 1) * KB, kk]
                    nc.tensor.matmul(
                        out=ups[ip * 32 : (ip + 1) * 32, c0 : c0 + KB],
                        lhsT=sel32_sb, rhs=rhs,
                        start=(kk == 0), stop=(kk == KB - 1),
                    )

        def chain_copy(ups, c0, w, rz, want_z, tag):
            """PSUM->SBUF bf16 state copy for cols [c0, c0+w); optional lazy
            scale rz and z-partial accumulation."""
            ns_ = sb.tile([P, w], BF, tag=f"ns{tag}", name=f"ns{tag}")
            zr = None
            if want_z or rz is not None:
                kw = {}
                if want_z:
                    zrt = sb.tile([P, 1], BF, tag=f"zr{tag}", name=f"zr{tag}")
                    kw["accum_out"] = zrt[:]
                    zr = zrt
                with nc.allow_low_precision("bf16 chain state"):
                    nc.vector.scalar_tensor_tensor(
                        out=ns_[:], in0=ups[:, c0 : c0 + w],
                        scalar=(1.0 if rz is None else rz),
                        in1=ones_pk[:, 0:w], op0=ALU.mult, op1=ALU.mult, **kw,
                    )
            else:
                nc.vector.tensor_copy(out=ns_[:], in_=ups[:, c0 : c0 + w])
            return ns_, zr

        def pair_renorm(zr, zslot, tag):
            """replicate+sum z partials into zb[:, zslot]; return 1/z AP."""
            nc.tensor.matmul(
                out=zb[:, zslot : zslot + 1], lhsT=sel3_sb, rhs=zr[:],
                start=True, stop=True,
            )
            rz = sb.tile([P, 1], FP, tag=f"rz{tag}", name=f"rz{tag}")
            nc.vector.reciprocal(out=rz[:], in_=zb[:, zslot : zslot + 1])
            return rz[:]

        # ---------------- main streamed loop ----------------
        def exp_chunks(nt):
            cs = (nt + CHOP - 1) // CHOP if CHOP else nt
            return [(a, min(a + cs, nt)) for a in range(0, nt, cs)]

        plan = [0]
        t_acc = min(FIRST, H) if FIRST else min(TC, H)
        while t_acc < H:
            plan.append(t_acc)
            t_acc += min(TC, H - t_acc)
        stF, stB = uf0, ub0
        pendF = pendB = None
        stU = us0
        rzF = rzB = None
        nF = nB = nU = 0
        for it, t0 in enumerate(plan):
            if 1 <= it <= nchunk - 1:
                load_eft_chunk(it)
            if it == nchunk:
                nc.sync.dma_start(out=gv[:], in_=gvals[:])
                nc.vector.tensor_reduce(out=sc_early[:], in_=gv[:], axis=AX.X, op=ALU.add)
            if it == len(plan) - 1:
                # all renorm z's are logged by now; fold the ln+sum here so it
                # overlaps the final slots instead of serializing in the tail
                nc.scalar.activation(out=lzb[:], in_=zb[:], func=AF.Ln)
                nc.vector.tensor_reduce(out=sFe[:], in_=lzb[:, 0:40], axis=AX.X, op=ALU.add)
                nc.vector.tensor_reduce(out=sBe[:], in_=lzb[:, 40:80], axis=AX.X, op=ALU.add)
            t_next = plan[it + 1] if it + 1 < len(plan) else H
            ntF = t_next - t0
            ntB = max(0, min(t_next, HB) - t0)
            ftileF = big.tile([P, TC * NF2], FP, tag="ftileF", name="ftileF")
            for c0, c1 in exp_chunks(ntF):
                nc.sync.dma_start(
                    out=ftileF[:, c0 * NF2 : c1 * NF2],
                    in_=fppF[:, :, t0 + c0 : t0 + c1, :].rearrange("i b t f -> (i b) (t f)"),
                )
            eF = ebig.tile([P, TC * NF2], BF, tag="eF", name="eF")
            for c0, c1 in exp_chunks(ntF):
                nc.scalar.activation(
                    out=eF[:, c0 * NF2 : c1 * NF2], in_=ftileF[:, c0 * NF2 : c1 * NF2],
                    func=AF.Exp, bias=cpb[0:P, :],
                )
            if ntB > 0:
                ftileB = big.tile([P, TC * NF2], FP, tag="ftileB", name="ftileB")
                for c0, c1 in exp_chunks(ntB):
                    nc.sync.dma_start(
                        out=ftileB[:, c0 * NF2 : c1 * NF2],
                        in_=fppB[:, :, t0 + c0 : t0 + c1, :].rearrange("i b t f -> (i b) (t f)"),
                    )
                eB = ebig.tile([P, TC * NF2], BF, tag="eB", name="eB")
                for c0, c1 in exp_chunks(ntB):
                    nc.scalar.activation(
                        out=eB[:, c0 * NF2 : c1 * NF2], in_=ftileB[:, c0 * NF2 : c1 * NF2],
                        func=AF.Exp, bias=cpb[0:P, :],
                    )

            for m in range(ntF):
                # ---- unary slot ----
                g = nU
                ef_sl = eft[:, g * BC : (g + 1) * BC]
                usm = sb.tile([UROW, BC], BF, tag="usm", name="usm")
                nc.vector.tensor_tensor(out=usm[:], in0=stU[:], in1=ef_sl, op=ALU.mult)
                rhs_u = usm
                nU += 1
                if nU % R == 0 and nU < SL:
                    # unary renorm: z from usm; scale usm before the matmul
                    zu_ps = ps1.tile([2, BC], FP, tag="pmisc", name="zu_ps")
                    nc.tensor.matmul(out=zu_ps[:], lhsT=uones[:], rhs=usm[:], start=True, stop=True)
                    zsl = zbufU[:, (nU // R) * BC : (nU // R + 1) * BC]
                    nc.vector.tensor_copy(out=zsl, in_=zu_ps[:])
                    rzu = sb.tile([2, BC], FP, tag="rzu", name="rzu")
                    nc.vector.reciprocal(out=rzu[:], in_=zu_ps[:])
                    rzu_b = sb.tile([2, BC], BF, tag="rzu_b", name="rzu_b")
                    nc.vector.tensor_copy(out=rzu_b[:], in_=rzu[:])
                    rzu_rep = ps1.tile([UROW, BC], FP, tag="pmisc", name="rzu_rep")
                    nc.tensor.matmul(out=rzu_rep[:], lhsT=usel[:], rhs=rzu_b[:], start=True, stop=True)
                    rzu_s = sb.tile([UROW, BC], BF, tag="rzu_s", name="rzu_s")
                    nc.scalar.activation(out=rzu_s[:], in_=rzu_rep[:], func=AF.Copy)
                    uscl = sb.tile([UROW, BC], BF, tag="uscl", name="uscl")
                    nc.vector.tensor_tensor(out=uscl[:], in0=usm[:], in1=rzu_s[:], op=ALU.mult)
                    rhs_u = uscl
                vu_ps = psU.tile([UROW, BC], FP, tag="vu", name="vu_ps")
                nc.tensor.matmul(out=vu_ps[:], lhsT=uw[:], rhs=rhs_u[:], start=True, stop=True)
                if UCOPY:
                    stU_sb = sb.tile([UROW, BC], BF, tag="stU", name="stU_sb")
                    nc.scalar.activation(out=stU_sb[:], in_=vu_ps[:], func=AF.Copy)
                    stU = stU_sb
                else:
                    stU = vu_ps

                # ---- deferred copy of fwd's previous PSUM state, then fwd step ----
                if pendF is not None:
                    stF, zrF = chain_copy(pendF[0], 0, KB, rzF, pendF[1], "F")
                    rzF = None
                    if pendF[1]:
                        rzF = pair_renorm(zrF, pendF[2], "F")
                    pendF = None
                wzF = ((nF + 1) % R == 0 and (nF + 1) < H)
                upsF = psF.tile([P, KB], FP, tag="upsF", name="upsF")
                ubF = stF[:, :].unsqueeze(1).broadcast_to([P, K, KB])
                pair_mm(eF, m, ubF, upsF, 0, "F")
                nF += 1
                pendF = (upsF, wzF, nF // R)

                # ---- same for bwd ----
                has_b = m < ntB
                if pendB is not None:
                    stB, zrB = chain_copy(pendB[0], 0, KB, rzB, pendB[1], "B")
                    rzB = None
                    if pendB[1]:
                        rzB = pair_renorm(zrB, pendB[2], "B")
                    pendB = None
                if has_b:
                    wzB = ((nB + 1) % R == 0 and (nB + 1) < HB)
                    upsB = psB.tile([P, KB], FP, tag="upsB", name="upsB")
                    ubB = stB[:, :].unsqueeze(1).broadcast_to([P, K, KB])
                    pair_mm(eB, m, ubB, upsB, 0, "B")
                    nB += 1
                    pendB = (upsB, wzB, 40 + nB // R)



        # ---------------- flush final deferred copies ----------------
        if pendF is not None:
            stF, _ = chain_copy(pendF[0], 0, KB, rzF, False, "F")
            pendF = None
        if pendB is not None:
            stB, _ = chain_copy(pendB[0], 0, KB, rzB, False, "B")
            pendB = None

        # ---------------- tails ----------------
        # pairwise meet: q[b] = sum_k uF[b,k]*uB[b,k]
        qm = sb.tile([P, KB], BF, tag="qm", name="qm")
        nc.vector.tensor_tensor(out=qm[:], in0=stF[:], in1=stB[:], op=ALU.mult)
        qr = sb.tile([P, 1], BF, tag="qr", name="qr")
        with nc.allow_low_precision("meet partial"):
            nc.vector.tensor_reduce(out=qr[:], in_=qm[:], axis=AX.X, op=ALU.add)
        qrep = ps1.tile([P, 1], FP, tag="pmisc", name="qrep")
        nc.tensor.matmul(out=qrep[:], lhsT=sel3_sb, rhs=qr[:], start=True, stop=True)
        lq = sb.tile([P, 1], FP, tag="lq", name="lq")
        nc.scalar.activation(out=lq[:], in_=qrep[:], func=AF.Ln)
        nc.vector.tensor_tensor(out=lq[:], in0=lq[:], in1=sFe[:], op=ALU.add)
        nc.vector.tensor_tensor(out=lq[:], in0=lq[:], in1=sBe[:], op=ALU.add)
        nc.vector.tensor_scalar(out=lq[:], in0=lq[:], scalar1=CP * (H + HB + 1), scalar2=None, op0=ALU.add)

        # unary tail: all per-b results assembled partition-major [32, 1]
        # via tiny PE transpose-matmuls (no DRAM bounces).
        efl = sb.tile([K, BC], FP, tag="efl", name="efl")
        nc.sync.dma_start(out=efl[:], in_=eflast[:])
        efl_e = sb.tile([K, BC], BF, tag="efl_e", name="efl_e")
        nc.scalar.activation(out=efl_e[:], in_=efl[:], func=AF.Exp)
        ustail = sb.tile([UROW, BC], BF, tag="ustail", name="ustail")
        nc.scalar.activation(out=ustail[:], in_=stU[:], func=AF.Copy)
        # bring bwd rows 32..32+K down to partitions 0..K via selector matmul
        usb_ps = ps1.tile([K, BC], FP, tag="pmisc", name="usb_ps")
        nc.tensor.matmul(out=usb_ps[:], lhsT=selDN_sb, rhs=ustail[:], start=True, stop=True)
        um = sb.tile([K, BC], BF, tag="um", name="um")
        nc.vector.tensor_tensor(out=um[:], in0=ustail[0:K, :], in1=efl_e[:], op=ALU.mult)
        nc.vector.tensor_tensor(out=um[:], in0=um[:], in1=usb_ps[:], op=ALU.mult)
        ones_k = sb.tile([K, 1], BF, tag="ones_k", name="ones_k")
        nc.vector.memset(ones_k[:], 1.0)
        au_t = ps1.tile([32, 1], FP, tag="pmisc", name="au_t")
        nc.tensor.matmul(out=au_t[:], lhsT=um[:], rhs=ones_k[:], start=True, stop=True)
        lau_t = sb.tile([32, 1], FP, tag="lau_t", name="lau_t")
        nc.scalar.activation(out=lau_t[:], in_=au_t[:], func=AF.Ln)
        lzU = per.tile([2, NR * BC], FP, tag="lzU", name="lzU")
        nc.scalar.activation(out=lzU[:], in_=zbufU[:], func=AF.Ln)
        sU = sb.tile([2, BC], FP, tag="sU", name="sU")
        nc.vector.tensor_reduce(
            out=sU[:], in_=lzU[:].rearrange("a (s b) -> a b s", b=BC),
            axis=AX.X, op=ALU.add,
        )
        sU_b = sb.tile([2, BC], BF, tag="sU_b", name="sU_b")
        nc.vector.tensor_copy(out=sU_b[:], in_=sU[:])
        su_t = ps1.tile([32, 1], FP, tag="pmisc", name="su_t")
        nc.tensor.matmul(out=su_t[:], lhsT=sU_b[:], rhs=ones2[:], start=True, stop=True)

        # score reduction + final combine (all [32, 1] partition-major)
        sc = sc_early
        res = sb.tile([BC, 1], FP, tag="res", name="res")
        nc.vector.tensor_tensor(out=res[:], in0=lq[0:32, :], in1=lau_t[:], op=ALU.add)
        nc.vector.tensor_tensor(out=res[:], in0=res[:], in1=su_t[:], op=ALU.add)
        nc.vector.tensor_scalar(out=res[:], in0=res[:], scalar1=CU * (2 * SL), scalar2=None, op0=ALU.add)
        nc.vector.tensor_tensor(out=res[:], in0=res[:], in1=sc[:], op=ALU.subtract)
        nc.sync.dma_start(out=nll[:], in_=res[:].rearrange("b o -> (b o)"))

    nc.compile()
    return nc


# ======================= host-side prep =======================

def prep_core_inputs(feats, fpp, transitions, tags, b0, BC, TT):
    H = TT // 2
    HB = TT - 1 - H
    fe = feats[b0 : b0 + BC]
    fp = fpp[b0 : b0 + BC]
    tg = tags[b0 : b0 + BC]
    fp4 = fp.reshape(BC, TT, K, K)          # [b, t, n(next), p(prev)]

    # fwd tiles: [b, i, t, (n, k8)] = fp4[b, t, n, i*KB+k8]
    fwd = fp4[:, 0:H].reshape(BC, H, K, NS, KB).transpose(3, 0, 1, 2, 4)
    fppF = np.ascontiguousarray(fwd.reshape(NS, BC, H, K * KB), np.float32)

    # bwd slot s holds matrix t = TT-2-s, transposed:
    # [b, i, s, (p, n8)] = fp4[b, t, n=i*KB+n8, p]
    bwd_t = fp4[:, H : TT - 1][:, ::-1]     # [b, s, n, p]
    bwd = bwd_t.reshape(BC, HB, NS, KB, K).transpose(2, 0, 1, 4, 3)
    fppB = np.ascontiguousarray(bwd.reshape(NS, BC, HB, K * KB), np.float32)

    # winit sliced to the (i, b) partition layout: [i*32+b, k8]
    winit = np.ascontiguousarray(
        fp4[:, TT - 1, STOP, :].reshape(BC, NS, KB).transpose(1, 0, 2).reshape(P, KB),
        np.float32)

    ftp2 = np.zeros((H, UROW, BC), np.float32)
    ftp2[1:, 0:K, :] = fe[:, 0 : H - 1].transpose(1, 2, 0)
    ftp2[:, 32 : 32 + K, :] = fe[:, TT - 1 : H - 1 : -1].transpose(1, 2, 0)
    eflast = np.ascontiguousarray(fe[:, H - 1, :].T, np.float32)

    tgi = np.asarray(tg, np.int64)
    te = np.concatenate([np.full((BC, 1), START, np.int64), tgi,
                         np.full((BC, 1), STOP, np.int64)], axis=1)
    nxt, prv = te[:, 1:], te[:, :-1]
    b_ = np.arange(BC)[:, None]
    t_ = np.arange(TT)[None, :]
    gvals = np.zeros((BC, 3 * TT + 4), np.float32)
    gvals[:, 0 : TT + 1] = transitions[nxt, prv]
    gvals[:, TT + 1 : 2 * TT + 1] = np.take_along_axis(
        fe, tgi[:, :, None], axis=2)[..., 0]
    gvals[:, 2 * TT + 1 : 3 * TT + 1] = fp4[b_, np.minimum(t_, TT - 2),
                                            nxt[:, 0:TT], prv[:, 0:TT]]
    gvals[:, 3 * TT] = fp4[np.arange(BC), TT - 1, STOP, tgi[:, -1]]
    gvals[:, 3 * TT - 1] = fp4[np.arange(BC), TT - 2, nxt[:, TT - 2], prv[:, TT - 2]]

    selDN = np.zeros((UROW, K), np.float32)
    for m in range(K):
        selDN[32 + m, m] = 1.0
    sel32 = np.zeros((P, 32), np.float32)
    sel3 = np.zeros((P, P), np.float32)
    for i in range(NS):
        for b in range(32):
            sel32[i * 32 + b, b] = 1.0
            for i2 in range(NS):
                sel3[i * 32 + b, i2 * 32 + b] = 1.0
    selpack = np.zeros((P, 32 + P + K), np.float32)
    selpack[:, 0:32] = sel32
    selpack[:, 32 : 32 + P] = sel3
    selpack[0:UROW, 32 + P :] = selDN
    transPK = np.zeros((UROW, K), np.float32)
    transPK[0:K, :] = transitions.T
    transPK[32 : 32 + K, :] = transitions

    return {
        "fppF": fppF,
        "fppB": fppB,
        "winit": winit,
        "ftp2": ftp2,
        "eflast": eflast,
        "transPK": transPK,
        "gvals": gvals,
        "selpack": selpack.astype(ml_dtypes.bfloat16),
    }


_NC_CACHE = {}


def get_nc(BC, TT, TC=20, R=32, COPY="dve", UCOPY=False, CHOP=5, FIRST=0, SBUFS=3, BIGB=2, EBIGB=3):
    key = (BC, TT, TC, R, COPY, UCOPY, CHOP, FIRST, SBUFS, BIGB, EBIGB)
    if key not in _NC_CACHE:
        _NC_CACHE[key] = build_kernel(BC=BC, TT=TT, TC=TC, R=R, COPY=COPY,
                                      UCOPY=UCOPY, CHOP=CHOP, FIRST=FIRST, SBUFS=SBUFS,
                                      BIGB=BIGB, EBIGB=EBIGB)
    return _NC_CACHE[key]


def kernel(feats, feats_pp, transitions, tags):
    feats = np.asarray(feats, np.float32)
    feats_pp = np.asarray(feats_pp, np.float32)
    transitions = np.asarray(transitions, np.float32)
    tags_np = np.asarray(tags)

    BC = B // NCORES
    nc = get_nc(BC, T)
    in_maps = [
        prep_core_inputs(feats, feats_pp, transitions, tags_np, c * BC, BC, T)
        for c in range(NCORES)
    ]
    r = run_bass_kernel_spmd(nc, in_maps, list(range(NCORES)))
    out = np.concatenate([r.results[c]["nll"] for c in range(NCORES)])
    return out.astype(np.float32)
